# revision 57
# baseline (speedup 1.0000x reference)
"""GCN-Tox21 GNN message-passing kernel for 8 Trainium2 NeuronCores (v2, fp8).

Strategy (graph/edge parallelism, fp8 DoubleRow pipeline):
  - Sort edges by destination; core k owns windows of 128 destination nodes
    (snake-dealt for balance). Per-window one-hot S/S_T tiles turn the
    dst-gather and segment-sum into PE matmuls (edges sorted by dst).
  - Everything per-edge runs in fp8e4 with dual-pumped matmuls
    (0.5 cycles/row):
      m1 = S_T.T@Q + eT.T@W1e' (one DoubleRow matmul: comb = [S_T; eT] fp8
           stationary pair, rhs = [Q_w ; W1e'] strided pair of qall_ext)
         + hsrcT.T@W1src''     (one DoubleRowSwInterleave matmul straight on
           the pair-interleaved fp8 dma_gather(transpose=True) output; gather
           indices are reversed per 128-block to compensate the ISA's
           column-reversed weight layout)
    then relu -> m1s fp8 (engines rotated Act/DVE/Pool), and the segment-sum
    runs as DoubleRow matmuls over PAIRS of subtiles sharing a window.
  - Node features h are stored scaled (1/4) in fp8 in DRAM; W1dst/W1src are
    pre-scaled x4 on the host. The e-embedding relu(ea@ee_w+b) is computed on
    the host and baked into the comb constant.
  - Per-window Q = h@W1dst'' is one SwInterleave matmul from hT, which is
    itself produced by a local dma_gather(transpose=True) from the core's own
    h rows (no PE transposes).
  - W2 stage: segsum copied PSUM->fp8, applied as DoubleRow chunk-pairs;
    BN folded via cb row + cnt rank-1 matmul (bf16, K=1).
  - AllGather per 4-window flush rebuilds replicated h (fp8, 4x less wire
    than the v1 zs scheme). Layer 2 output (128-dim) stays bf16 for pooling.
  - Mean-pool + FC + sigmoid unchanged from v1.
"""

import numpy as np
import ml_dtypes

import concourse.bacc as bacc
import concourse.tile as tile
from concourse import mybir, bass_utils
from concourse.masks import make_identity

BF16 = mybir.dt.bfloat16
F8 = mybir.dt.float8e4
F32 = mybir.dt.float32
RELU = mybir.ActivationFunctionType.Relu
COPY = mybir.ActivationFunctionType.Copy
DR = mybir.MatmulPerfMode.DoubleRow
SWI = mybir.MatmulPerfMode.DoubleRowSwInterleave

N_CORES = 8
BN_EPS = 1e-5
G_REAL = 512
F_NODE, F_EDGE, H, EH = 32, 8, 256, 16
OUT_DIMS = (256, 256, 128)
EG = 512
SCL = 4.0
SCRATCH = 16384
GATHER_N = 512

f8np = ml_dtypes.float8_e4m3


def _bf(a):
    return np.ascontiguousarray(np.asarray(a).astype(ml_dtypes.bfloat16))


def _f8(a):
    return np.ascontiguousarray(np.asarray(a).astype(f8np))


def _f32(a):
    return np.ascontiguousarray(np.asarray(a).astype(np.float32))


def _wrap_idx(idx):
    """int16 index layout for dma_gather: index i at [i % 16, i // 16],
    replicated across the 8 partition groups."""
    assert len(idx) % 16 == 0
    w = idx.astype(np.int16).reshape(-1, 16).T
    return np.ascontiguousarray(np.tile(w, (8, 1)))


def _rev128(idx):
    """reverse order within each 128-block (SwInterleave column reversal)."""
    return np.ascontiguousarray(idx.reshape(-1, 128)[:, ::-1].reshape(-1))


class Plan:
    """Host-side preprocessing: sharding layout + per-core input tensors."""

    def __init__(self, inputs, G):
        x = np.asarray(inputs["x"]).astype(np.float32)
        N = x.shape[0]
        self.N, self.G = N, G
        self.N_pad = ((N + N_CORES * 128 - 1) // (N_CORES * 128)) * (N_CORES * 128)
        self.NPC = self.N_pad // N_CORES
        self.W = self.NPC // 128
        assert G % N_CORES == 0
        self.GPC = G // N_CORES

        edge_index = np.asarray(inputs["edge_index"]).astype(np.int64)
        src, dst = edge_index[0].astype(np.int32), edge_index[1].astype(np.int32)
        batch = np.asarray(inputs["batch"]).astype(np.int32)
        edge_attr = np.asarray(inputs["edge_attr"]).astype(np.float32)

        ee_w, ee_b = _f32(inputs["ee_w"]), _f32(inputs["ee_b"])
        e_full = np.maximum(edge_attr @ ee_w.T + ee_b, 0.0)  # [E, EH] host e

        order = np.argsort(dst, kind="stable")
        s_dst, s_src = dst[order], src[order]
        s_e = e_full[order]

        bounds = np.searchsorted(s_dst, np.arange(0, self.N_pad + 1, 128), "left")
        cnt_all = bounds[1:] - bounds[:-1]
        t_all = np.maximum(1, -(-cnt_all // 128))
        NW = len(t_all)
        order_w = np.argsort(-t_all, kind="stable")
        slots = [[] for _ in range(N_CORES)]
        for pos, win in enumerate(order_w):
            rnd, r = divmod(pos, N_CORES)
            k = r if rnd % 2 == 0 else N_CORES - 1 - r
            slots[k].append(int(win))
        self.slots = slots
        owner = np.zeros(NW, np.int64)
        slot = np.zeros(NW, np.int64)
        for k, lst in enumerate(slots):
            for j, winid in enumerate(lst):
                owner[winid] = k
                slot[winid] = j
        T_w = np.array([max(t_all[slots[k][j]] for k in range(N_CORES))
                        for j in range(self.W)])
        while T_w.sum() % (EG // 128) != 0:
            T_w[-1] += 1
        self.T_w = [int(t) for t in T_w]
        self.T_tot = int(T_w.sum())
        self.ET = self.T_tot * 128

        # flush-major row permutation: node n -> row of h_full.
        def rowperm(n):
            win = n // 128
            p = n % 128
            m = slot[win] // 4
            return (m * (N_CORES * 512) + owner[win] * 512
                    + (slot[win] % 4) * 128 + p)

        cnt = np.bincount(dst, minlength=self.N_pad).astype(np.float32)
        invc_full = 1.0 / np.maximum(cnt, 1.0)
        cntp_full = np.maximum(cnt, 1.0)
        gcnt = np.bincount(batch, minlength=G).astype(np.float32)
        ginv_full = 1.0 / np.maximum(gcnt, 1.0)

        lo_k = [int(np.searchsorted(batch, k * self.GPC, "left")) for k in range(N_CORES)]
        hi_k = [int(np.searchsorted(batch, (k + 1) * self.GPC, "left")) for k in range(N_CORES)]
        self.TP = max(1, max(-(-(h - l) // 128) for l, h in zip(lo_k, hi_k)))
        self.NPOOL = self.TP * 128

        self.per_core = []
        for k in range(N_CORES):
            d = {}
            gi_src = np.zeros(self.ET, np.int32)
            e_pad = np.zeros((self.ET, EH), np.float32)
            S = np.zeros((128, self.ET), np.float32)
            pos = 0
            for w in range(self.W):
                base = slots[k][w] * 128
                lo = np.searchsorted(s_dst, base, "left")
                hi = np.searchsorted(s_dst, base + 128, "left")
                n = hi - lo
                sl = slice(pos, pos + n)
                gi_src[sl] = s_src[lo:hi]
                e_pad[sl] = s_e[lo:hi]
                loc = (s_dst[lo:hi] - base).astype(np.int64)
                e_ids = np.arange(pos, pos + n)
                S[e_ids % 128, (e_ids // 128) * 128 + loc] = 1.0
                pos += self.T_w[w] * 128
            assert pos == self.ET

            # src gather: global zs_full rows (plain order; the DoubleRow
            # identity-add is not SwInterleave, so no reversal)
            d["gidx_src"] = _wrap_idx(rowperm(gi_src))
            d["S8"] = _f8(S)
            # comb: [n, t, 0, r] = S_T ; [0:EH, t, 1, r] = e.T ; [EH, t, 1, r]=1
            comb = np.zeros((128, self.T_tot, 2, 128), np.float32)
            ST = S.reshape(128, self.T_tot, 128).transpose(2, 1, 0)
            comb[:, :, 0, :] = ST
            eT = e_pad.reshape(self.T_tot, 128, EH).transpose(2, 0, 1)
            comb[0:EH, :, 1, :] = eT
            comb[EH, :, 1, :] = 1.0
            d["comb"] = _f8(comb)

            win_ids = np.array(slots[k])
            nidx = win_ids[None, :] * 128 + np.arange(128)[:, None]  # [128, W]
            d["invc"] = _f32(invc_full[nidx])
            d["invcs"] = _f32(invc_full[nidx] / SCL)
            d["cntrow"] = _bf(cntp_full[nidx.T.reshape(-1)].reshape(1, self.NPC))
            lo, hi = lo_k[k], hi_k[k]
            pidx = np.zeros(self.NPOOL, np.int32)
            pidx[: hi - lo] = np.arange(lo, hi)
            d["pool_idx"] = _wrap_idx(rowperm(pidx))
            S2 = np.zeros((128, self.TP * self.GPC), np.float32)
            pb = batch[lo:hi] - k * self.GPC
            e_ids = np.arange(hi - lo)
            S2[e_ids % 128, (e_ids // 128) * self.GPC + pb] = 1.0
            d["S2"] = _bf(S2)
            d["ginv"] = _f32(ginv_full[k * self.GPC:(k + 1) * self.GPC].reshape(self.GPC, 1))
            self.per_core.append(d)

        sh = {}
        x_pad = np.zeros((self.N_pad, F_NODE), np.float32)
        x_pad[:N] = x
        xT_full = np.concatenate([x_pad.T, np.ones((1, self.N_pad), np.float32)], 0)
        for k in range(N_CORES):
            win_ids = np.array(slots[k])
            cols = (win_ids[:, None] * 128 + np.arange(128)[None, :]).reshape(-1)
            self.per_core[k]["xT_own"] = _bf(xT_full[:, cols])
        ne_w, ne_b = _f32(inputs["ne_w"]), _f32(inputs["ne_b"])
        sh["ne_wT"] = _bf(np.concatenate([ne_w.T, ne_b[None, :]], 0))

        in_dim = H
        self.layer_dims = []
        for i, out_dim in enumerate(OUT_DIMS):
            w1 = _f32(inputs[f"c{i}_w1"]); b1 = _f32(inputs[f"c{i}_b1"])
            w2 = _f32(inputs[f"c{i}_w2"]); b2 = _f32(inputs[f"c{i}_b2"])
            g = _f32(inputs[f"bn{i}_g"]); bb = _f32(inputs[f"bn{i}_b"])
            rm = _f32(inputs[f"bn{i}_m"]); rv = _f32(inputs[f"bn{i}_v"])
            A = g / np.sqrt(rv + BN_EPS)
            F_in = in_dim
            F_mid = 2 * out_dim
            MC = F_mid // 128
            W1dst = w1[:, :F_in]            # [F_mid, F_in]
            W1src = w1[:, F_in:2 * F_in]
            W1e = w1[:, 2 * F_in:]          # [F_mid, EH]
            # chunk-major (feature j*128+p on partition p, half j), x SCL
            sh[f"wdst8_{i}"] = _f8(
                (W1dst.T * SCL).reshape(2, 128, F_mid).transpose(1, 0, 2))
            sh[f"wsrc8_{i}"] = _f8(
                (W1src.T * SCL).reshape(2, 128, F_mid).transpose(1, 0, 2))
            we8 = np.zeros((128, F_mid), np.float32)
            we8[0:EH] = W1e.T
            we8[EH] = b1
            sh[f"we8_{i}"] = _f8(we8)
            w2A = (w2 * A[:, None]).T      # [F_mid, F_out]
            w28 = np.zeros((128, MC // 2, 2, out_dim), np.float32)
            for jp in range(MC // 2):
                for ii in range(2):
                    w28[:, jp, ii, :] = w2A[(2 * jp + ii) * 128:(2 * jp + ii + 1) * 128, :]
            sh[f"w28_{i}"] = _f8(w28)
            sh[f"cbrow_{i}"] = _bf((b2 * A + bb - rm * A)[None, :])
            self.layer_dims.append((F_in, F_mid, out_dim))
            in_dim = out_dim

        ip = np.zeros((128, 2, 128), np.float32)
        ip[:, 0, :] = np.eye(128)
        ip[:, 1, :] = np.eye(128)
        sh["identpair"] = _f8(ip)

        fc_w, fc_b = _f32(inputs["fc_w"]), _f32(inputs["fc_b"])
        self.F_FC = fc_w.shape[0]
        sh["fc_wT"] = _bf(fc_w.T)
        sh["fcb_bc"] = _f32(np.tile(fc_b[None, :], (self.GPC, 1)))
        self.shared = sh

    def in_maps(self):
        return [{**self.shared, **self.per_core[k]} for k in range(N_CORES)]


def build_program(plan: Plan, n_cores=N_CORES, debug_no_collective=False,
                  debug_stage=9, repeats=1):
    nc = bacc.Bacc("TRN2", target_bir_lowering=False, debug=False,
                   num_devices=n_cores, dynamic_dma_scratch_size=SCRATCH)

    ET, T_w, W, NPC, TP, GPC = plan.ET, plan.T_w, plan.W, plan.NPC, plan.TP, plan.GPC
    N_pad, NPOOL, F_FC = plan.N_pad, plan.NPOOL, plan.F_FC

    sample = plan.in_maps()[0]
    t_in = {name: nc.dram_tensor(name, list(arr.shape),
                                 mybir.dt.from_np(arr.dtype), kind="ExternalInput")
            for name, arr in sample.items()}
    out_part = nc.dram_tensor("out_part", [GPC, F_FC], F32, kind="ExternalOutput")

    n_batches = ET // EG

    # subtile -> window mapping (static)
    sub_window, sub_tloc, sub_last = [], [], []
    for w in range(W):
        for t in range(T_w[w]):
            sub_window.append(w)
            sub_tloc.append(t)
            sub_last.append(t == T_w[w] - 1)

    with tile.TileContext(nc) as tc:
        with (
            tc.tile_pool(name="const", bufs=1) as cpool,
            tc.tile_pool(name="sbuf", bufs=2) as spool,
            tc.tile_pool(name="gath", bufs=3) as gpool,
            tc.tile_pool(name="m1sb", bufs=8) as m1pool,
            tc.tile_pool(name="psum", bufs=2, space="PSUM") as ppool,
            tc.tile_pool(name="dram", bufs=1, space="DRAM") as dpool,
        ):
            def _body():
                def load_const(name, tag=None):
                    arr = sample[name]
                    t = cpool.tile(list(arr.shape), mybir.dt.from_np(arr.dtype),
                                   tag=tag or name)
                    nc.sync.dma_start(out=t[:], in_=t_in[name][:])
                    return t

                # stage-A-critical loads first
                xo_all = spool.tile([F_NODE + 1, NPC], BF16, tag="xoall", bufs=1)
                nc.sync.dma_start(out=xo_all[:], in_=t_in["xT_own"][:])
                ne_wT_t = load_const("ne_wT")
                ident = cpool.tile([128, 128], BF16, tag="ident")
                make_identity(nc, ident[:])

                gidx_src = load_const("gidx_src")
                invc_t = load_const("invc")
                invcs_t = load_const("invcs")
                cntrow_t = load_const("cntrow")
                identpair_t = load_const("identpair")

                wdst8_t = [load_const(f"wdst8_{i}") for i in range(3)]
                wsrc8_t = [load_const(f"wsrc8_{i}") for i in range(3)]
                w28_t = [load_const(f"w28_{i}") for i in range(3)]
                cbrow_t = [load_const(f"cbrow_{i}") for i in range(3)]

                # allocated here; loaded after stage A is emitted (SP queue
                # order) so layer 0's h writes aren't stuck behind 10.4MB
                T_tot = plan.T_tot
                comb_t = cpool.tile([128, T_tot, 2, 128], F8, tag="comb")
                S8_t = cpool.tile([128, ET], F8, tag="S8")

                def load_bulk_consts():
                    NCH = 8
                    for q in range(NCH):
                        t0 = (T_tot * q) // NCH
                        t1 = (T_tot * (q + 1)) // NCH
                        nc.sync.dma_start(out=comb_t[:, t0:t1, :, :],
                                          in_=t_in["comb"][:, t0:t1, :, :])
                        nc.sync.dma_start(out=S8_t[:, t0 * 128:t1 * 128],
                                          in_=t_in["S8"][:, t0 * 128:t1 * 128])

                # per-layer hT (chunk-major [128, W, 2, 128]) / qall /
                # zs-staging tiles, filled by the producing layer
                layer_F_mid = [d[1] for d in plan.layer_dims]
                hT_t = [spool.tile([128, W, 2, 128], F8, tag=f"hT{i}", bufs=1,
                                   name=f"hT{i}") for i in range(3)]
                qall_t = [spool.tile([128, W + 1, layer_F_mid[i]], F8,
                                     tag=f"qall{i}", bufs=1, name=f"qall{i}")
                          for i in range(3)]
                zstage_t = [spool.tile([128, 4, layer_F_mid[i]], F8,
                                       tag=f"zstage{i}", bufs=2,
                                       name=f"zstage{i}") for i in range(3)]
                for i in range(3):
                    nc.sync.dma_start(out=qall_t[i][:, W, :],
                                      in_=t_in[f"we8_{i}"][:])

                # ---------- DRAM buffers ----------
                zs_own = [dpool.tile([NPC, layer_F_mid[i]], F8,
                                     tag=f"zsown{i}", name=f"zs_own{i}")
                          for i in range(3)]
                zs_full = [dpool.tile([N_pad, layer_F_mid[i]], F8,
                                      tag=f"zsfull{i}", name=f"zs_full{i}")
                           for i in range(3)]
                F_last = plan.layer_dims[-1][2]
                h_own3 = dpool.tile([NPC, F_last], BF16, tag="hown3",
                                    name="h_own3")
                h_full3 = dpool.tile([n_cores * NPC, F_last], BF16,
                                     tag="hfull3", name="h_full3")

                def allgather(src, dst, m):
                    if debug_no_collective:
                        cp = spool.tile([128, 128], src.dtype, tag="dbgcp")
                        nc.sync.dma_start(
                            out=cp[:], in_=src[m * 512:m * 512 + 128, 0:128])
                        nc.sync.dma_start(
                            out=dst[m * n_cores * 512:m * n_cores * 512 + 128,
                                    0:128], in_=cp[:])
                    else:
                        nc.gpsimd.collective_compute(
                            "AllGather", mybir.AluOpType.bypass,
                            ins=[src[m * 512:(m + 1) * 512, :].opt()],
                            outs=[dst[m * n_cores * 512:
                                      (m + 1) * n_cores * 512, :].opt()],
                            replica_groups=[list(range(n_cores))])

                def prep_window(li, wp):
                    """Q and zs projections for window wp of layer li's input
                    (consumes hT[li], produced a couple windows earlier)."""
                    Fm = layer_F_mid[li]
                    qtp = ppool.tile([128, Fm], F32, tag="m2")
                    nc.tensor.matmul(
                        out=qtp[:], lhsT=hT_t[li][:, wp, :, :],
                        rhs=wdst8_t[li][:], start=True, stop=True,
                        perf_mode=DR, skip_group_check=True)
                    if wp % 2 == 0:
                        nc.scalar.activation(out=qall_t[li][:, wp, :],
                                             in_=qtp[:], func=COPY)
                    else:
                        nc.vector.tensor_copy(out=qall_t[li][:, wp, :],
                                              in_=qtp[:])
                    ztp = ppool.tile([128, Fm], F32, tag="m2")
                    nc.tensor.matmul(
                        out=ztp[:], lhsT=hT_t[li][:, wp, :, :],
                        rhs=wsrc8_t[li][:], start=True, stop=True,
                        perf_mode=DR, skip_group_check=True)
                    zdst = zstage_t[li][:, wp % 4, :]
                    if wp % 2 == 0:
                        nc.vector.tensor_copy(out=zdst, in_=ztp[:])
                    else:
                        nc.scalar.activation(out=zdst, in_=ztp[:], func=COPY)
                    if wp % 4 == 3:
                        m = wp // 4
                        nc.sync.dma_start(
                            out=zs_own[li][m * 512:(m + 1) * 512, :]
                                .rearrange("(a p) c -> p a c", a=4),
                            in_=zstage_t[li][:])
                        allgather(zs_own[li], zs_full[li], m)

                PREP_LAGW = 2

                def produce_hT(li, w, hsbA):
                    """bf16 transposes -> fp8 chunk-major hT[li] window w,
                    then lagged Q/zs prep."""
                    tp2 = ppool.tile([128, 2, 128], BF16, tag="m2")
                    for c in range(2):
                        nc.tensor.transpose(out=tp2[:, c, :],
                                            in_=hsbA[:, c * 128:(c + 1) * 128],
                                            identity=ident[:])
                    if w % 2 == 0:
                        nc.vector.tensor_copy(out=hT_t[li][:, w, :, :],
                                              in_=tp2[:])
                    else:
                        nc.scalar.activation(out=hT_t[li][:, w, :, :],
                                             in_=tp2[:], func=COPY)
                    if w - PREP_LAGW >= 0:
                        prep_window(li, w - PREP_LAGW)
                    if w == W - 1:
                        for wp in range(W - PREP_LAGW, W):
                            prep_window(li, wp)

                # ---------- stage A: h0 = relu(x @ ne_w.T + ne_b)/SCL ----------
                for w in range(W):
                    ps = ppool.tile([128, H], F32, tag="m2")
                    nc.tensor.matmul(out=ps[:], lhsT=xo_all[:, w * 128:(w + 1) * 128],
                                     rhs=ne_wT_t[:], start=True, stop=True)
                    hsbA = spool.tile([128, H], BF16, tag="hsbA", bufs=3)
                    nc.scalar.activation(out=hsbA[:], in_=ps[:], func=RELU,
                                         scale=1.0 / SCL)
                    produce_hT(0, w, hsbA)

                # bulk + tail-stage constants stream in behind stage A
                load_bulk_consts()
                pool_idx = load_const("pool_idx")
                S2_t = load_const("S2")
                ginv_t = load_const("ginv")
                fc_wT_t = load_const("fc_wT")
                fcb_t = load_const("fcb_bc")

                if debug_stage < 3:
                    return

                # ---------- conv layers ----------
                for li, (F_in, F_mid, F_out) in enumerate(plan.layer_dims):
                    if debug_stage == 35 and li > 0:
                        break
                    MC = F_mid // 128
                    qall = qall_t[li]
                    # zero slot (index 8) of the two rotating gather tiles
                    zsgA = spool.tile([128, 9, F_mid], F8, tag="zsgA", bufs=1,
                                      name=f"zsgA{li}")
                    zsgB = spool.tile([128, 9, F_mid], F8, tag="zsgB", bufs=1,
                                      name=f"zsgB{li}")
                    nc.gpsimd.memset(zsgA[:, 8, :], 0.0)
                    nc.gpsimd.memset(zsgB[:, 8, :], 0.0)
                    zsg_tiles = [zsgA, zsgB]
                    if debug_stage == 33:
                        break

                    # (c) per-edge pipeline
                    node_ps = None
                    m1s_cur = None
                    zsg_cur = None

                    def do_segsum(t_glob, w, m1s, npair):
                        nonlocal node_ps
                        t0 = t_glob - (npair - 1)
                        first = sub_tloc[t0] == 0
                        last = sub_last[t_glob]
                        if first:
                            node_ps = ppool.tile([128, MC, 128], F32,
                                                 tag="node", bufs=2)
                        for fc in range(MC):
                            if npair == 2:
                                nc.tensor.matmul(
                                    out=node_ps[:, fc, :],
                                    lhsT=m1s[:, :, fc * 128:(fc + 1) * 128],
                                    rhs=S8_t[:, t0 * 128:(t0 + 2) * 128]
                                        .rearrange("p (two n) -> p two n", two=2),
                                    start=first, stop=last and fc == MC - 1,
                                    perf_mode=DR, skip_group_check=True)
                            else:
                                nc.tensor.matmul(
                                    out=node_ps[:, fc, :],
                                    lhsT=m1s[:, 0, fc * 128:(fc + 1) * 128],
                                    rhs=S8_t[:, t0 * 128:(t0 + 1) * 128],
                                    start=first, stop=last and fc == MC - 1,
                                    skip_group_check=True)

                    def window_tail(w):
                        ntsb = spool.tile([128, MC, 128], F8, tag="ntsb", bufs=2)
                        if w % 2 == 0:
                            nc.vector.tensor_copy(out=ntsb[:], in_=node_ps[:])
                        else:
                            nc.scalar.activation(out=ntsb[:], in_=node_ps[:],
                                                 func=COPY)
                        out2 = ppool.tile([128, F_out], F32, tag="m2")
                        nc.tensor.matmul(
                            out=out2[:],
                            lhsT=cntrow_t[0:1, w * 128:(w + 1) * 128],
                            rhs=cbrow_t[li][:], start=True, stop=False,
                            skip_group_check=True)
                        for jp in range(MC // 2):
                            nc.tensor.matmul(
                                out=out2[:], lhsT=ntsb[:, 2 * jp:2 * jp + 2, :],
                                rhs=w28_t[li][:, jp, :, :],
                                start=False, stop=jp == MC // 2 - 1,
                                perf_mode=DR, skip_group_check=True)
                        if li < 2:
                            hsbA = spool.tile([128, F_out], BF16, tag="hsbA",
                                              bufs=3)
                            if w % 2 == 0:
                                nc.scalar.activation(out=hsbA[:], in_=out2[:],
                                                     func=RELU,
                                                     scale=invcs_t[:, w:w + 1])
                            else:
                                nc.vector.tensor_scalar(
                                    out=hsbA[:], in0=out2[:],
                                    scalar1=invcs_t[:, w:w + 1], scalar2=0.0,
                                    op0=mybir.AluOpType.mult,
                                    op1=mybir.AluOpType.max)
                            produce_hT(li + 1, w, hsbA)
                        else:
                            hsb3 = spool.tile([128, F_out], BF16, tag="hsb3",
                                              bufs=3)
                            nc.scalar.activation(out=hsb3[:], in_=out2[:],
                                                 func=RELU,
                                                 scale=invc_t[:, w:w + 1])
                            nc.sync.dma_start(
                                out=h_own3[w * 128:(w + 1) * 128, :],
                                in_=hsb3[:])
                            if w % 4 == 3:
                                allgather(h_own3, h_full3, w // 4)

                    def flush_pend(items):
                        for (t_glob, w, m1s, npair, tail) in items:
                            if npair:
                                do_segsum(t_glob, w, m1s, npair)
                            if tail:
                                window_tail(w)

                    def emit_src_gather(g0):
                        ng = min(1024, ET - g0 * 512)
                        t = zsg_tiles[(g0 // 2) % 2]
                        nc.gpsimd.dma_gather(
                            t[:, 0:ng // 128, :], zs_full[li][:, :],
                            gidx_src[:, g0 * 32:g0 * 32 + ng // 16],
                            ng, ng, F_mid, transpose=False)
                        return t

                    LAG = 2  # batches between relu emission and segsum/tail
                    pend_q = []
                    next_gather = emit_src_gather(0)
                    for g in range(n_batches):
                        if g % 2 == 0:
                            zsg_cur = next_gather
                            if g + 2 < n_batches:
                                next_gather = emit_src_gather(g + 2)
                        new_pend = []
                        for s in range(4):
                            t_glob = g * 4 + s
                            w = sub_window[t_glob]
                            t_loc = sub_tloc[t_glob]
                            half = t_loc % 2
                            if half == 0:
                                m1s_cur = m1pool.tile([128, 2, F_mid], F8,
                                                      tag="m1s")
                            m1p = ppool.tile([128, F_mid], F32, tag="m1",
                                             bufs=4)
                            nc.tensor.matmul(
                                out=m1p[:],
                                lhsT=comb_t[:, t_glob, :, :],
                                rhs=qall[:, w::(W - w), :],
                                start=True, stop=False,
                                perf_mode=DR, skip_group_check=True)
                            s8 = (g % 2) * 4 + s
                            nc.tensor.matmul(
                                out=m1p[:],
                                lhsT=identpair_t[:],
                                rhs=zsg_cur[:, s8::(8 - s8), :],
                                start=False, stop=True,
                                perf_mode=DR, skip_group_check=True)
                            if t_glob % 2 == 0:
                                nc.scalar.activation(
                                    out=m1s_cur[:, half, :], in_=m1p[:],
                                    func=RELU)
                            else:
                                nc.vector.tensor_scalar_max(
                                    out=m1s_cur[:, half, :], in0=m1p[:],
                                    scalar1=0.0)
                            npair = 0
                            if t_loc % 2 == 1:
                                npair = 2
                            elif sub_last[t_glob]:
                                npair = 1
                            if npair:
                                new_pend.append((t_glob, w, m1s_cur, npair,
                                                 sub_last[t_glob]))
                        pend_q.append(new_pend)
                        if len(pend_q) > LAG:
                            flush_pend(pend_q.pop(0))
                    for items in pend_q:
                        flush_pend(items)
                    pend_q = []

                # ---------- pooling + FC + sigmoid ----------
                if debug_stage < 5:
                    return
                hp = spool.tile([128, TP, F_last], BF16, tag="hp", bufs=1)
                for p0 in range(0, TP, 4):
                    pn = min(4, TP - p0)
                    nc.gpsimd.dma_gather(
                        hp[:, p0:p0 + pn, :],
                        h_full3[:, :],
                        pool_idx[:, p0 * 8:(p0 + pn) * 8],
                        pn * 128, pn * 128, F_last, transpose=False)
                pool_ps = ppool.tile([GPC, F_last], F32, tag="m1", bufs=4)
                for t in range(TP):
                    nc.tensor.matmul(out=pool_ps[:],
                                     lhsT=S2_t[:, t * GPC:(t + 1) * GPC],
                                     rhs=hp[:, t, :], start=(t == 0),
                                     stop=(t == TP - 1))
                pooled_sb = spool.tile([GPC, F_last], BF16, tag="pooled")
                nc.scalar.activation(out=pooled_sb[:], in_=pool_ps[:],
                                     func=COPY, scale=ginv_t[:])
                ptr_ps = ppool.tile([F_last, GPC], BF16, tag="m2")
                nc.tensor.transpose(out=ptr_ps[:], in_=pooled_sb[:],
                                    identity=ident[0:GPC, 0:GPC])
                ptr_sb = spool.tile([F_last, GPC], BF16, tag="ptrsb")
                nc.vector.tensor_copy(out=ptr_sb[:], in_=ptr_ps[:])
                fc_ps = ppool.tile([GPC, F_FC], F32, tag="node")
                nc.tensor.matmul(out=fc_ps[:], lhsT=ptr_sb[:], rhs=fc_wT_t[:],
                                 start=True, stop=True)
                logit = spool.tile([GPC, F_FC], F32, tag="logit")
                nc.vector.tensor_tensor(out=logit[:], in0=fc_ps[:], in1=fcb_t[:],
                                        op=mybir.AluOpType.add)
                nc.sync.dma_start(out=out_part[:], in_=logit[:])

            for _r in range(repeats):
                _body()

    nc.compile()
    return nc


_CACHE = {}


def run(inputs, G=G_REAL):
    plan = Plan(inputs, G)
    key = (plan.N, plan.G, plan.TP, tuple(plan.T_w))
    if key not in _CACHE:
        _CACHE[key] = build_program(plan)
    nc = _CACHE[key]
    res = bass_utils.run_bass_kernel_spmd(nc, plan.in_maps(),
                                          core_ids=list(range(N_CORES)))
    logits = np.concatenate([res.results[k]["out_part"] for k in range(N_CORES)], 0)
    out = 1.0 / (1.0 + np.exp(-logits.astype(np.float64)))
    return np.ascontiguousarray(out.astype(np.float32))


def kernel(**inputs) -> np.ndarray:
    return run(inputs, G=G_REAL)
